# revision 1
# baseline (speedup 1.0000x reference)
"""Trainium2 Bass kernel for nn_DecoderMixer (L=13, B=4, T=1024, C=1024, H=16).

v5 = v4 plus 3-term fp8 (e4m3 DoubleRow) Q/K/V projections:
  x = xh + xl, W = wh + wl (unscaled residuals, all terms share one PSUM
  accumulation chain); x@W ~= xh@wh + xh@wl + xl@wh. O stays f32r.

Sharding: data-parallel over the fused B*T axis - 8 cores x 512 rows.
RoPE folded into weights host-side; only the last query position is used.
"""

import numpy as np

import concourse.tile as tile
from concourse import bacc, mybir

L, B, T, C = 13, 4, 1024, 1024
H, D = 16, 64
N_CORES = 8
NPC = (B * T) // N_CORES   # 512 rows per core
CHUNK = 128
NCHUNK = NPC // CHUNK      # 4
CI = C // 128              # 8 contraction tiles
ROPE_BASE = 10000.0

F32 = mybir.dt.float32
F32R = mybir.dt.float32r
F8 = mybir.dt.float8e4
DR = mybir.MatmulPerfMode.DoubleRow
GG = 4                      # 256-wide contraction groups
SX, SW = 16.0, 256.0        # fp8 pre-scales for x and W
QSCALE = 1.0 / (SX * SW) ** 2

L_ORDER = [12] + list(range(12))

_CACHED_NC = None
_CACHED_RUNNER = None


def _emit(tc, aps):
    nc = tc.nc
    xt, wkt, wvt, wqt, wot, ident, chain, out = (
        aps["xt"], aps["wkt"], aps["wvt"], aps["wqt"], aps["wot"],
        aps["ident"], aps["chain"], aps["out"],
    )

    with (
        tc.tile_pool(name="wk", bufs=2) as wk_pool,
        tc.tile_pool(name="x", bufs=2) as x_pool,
        tc.tile_pool(name="res", bufs=1) as res_pool,
        tc.tile_pool(name="small", bufs=4) as small_pool,
        tc.tile_pool(name="p", bufs=2) as p_pool,
        tc.tile_pool(name="o", bufs=2) as o_pool,
        tc.tile_pool(name="ps", bufs=4, space="PSUM") as ps_pool,
    ):
        # chain dummy: serializes reps in the timing harness
        ch_sb = res_pool.tile([128, 2], F32, tag="chain")
        nc.sync.dma_start(ch_sb[:], chain[0:128, 0:2])

        # ---- startup: wq + first-layer x first (split across two issue
        # queues so the Q projection can start within a few us) ----
        # fp8 tiles: [128, 2(hi/lo), GG, 2(pair), *] ; dram layouts match.
        wq_sb = wk_pool.tile([128, 2, GG, 2, C], F8, tag="w", name="wq")
        l0 = L_ORDER[0]
        x12_sb = x_pool.tile([128, 2, GG, 2, NPC], F8, tag="x", name="x12")
        for t in range(2):
            for gg in range(GG):
                nc.sync.dma_start(wq_sb[:, t, gg], wqt[t, gg])
                nc.scalar.dma_start(x12_sb[:, t, gg], xt[l0, t, gg])
        wk12_sb = wk_pool.tile([128, 2, GG, 2, C], F8, tag="w", name="wk12")
        for t in range(2):
            for gg in range(GG):
                nc.scalar.dma_start(wk12_sb[:, t, gg], wkt[l0, t, gg])

        q_sb = res_pool.tile([128, NCHUNK, C], F32, tag="q")
        num_sb = res_pool.tile([128, NCHUNK, H, D], F32, tag="num")
        e_all = res_pool.tile([128, NCHUNK, L, H], F32, tag="e_all")
        nc.gpsimd.memset(num_sb[:], 0.0)

        def mm3(ps_t, x_t, w_t, cs, hs):
            """12-matmul 3-term fp8 chain into ps_t[:, hs]."""
            first = True
            for xi, wi in ((0, 0), (0, 1), (1, 0)):
                for gg in range(GG):
                    nc.tensor.matmul(
                        ps_t[:, hs], x_t[:, xi, gg, :, cs], w_t[:, wi, gg, :, hs],
                        start=first, stop=(xi == 1 and gg == GG - 1),
                        perf_mode=DR,
                    )
                    first = False

        # Q projection for all chunks (covers wk12/wv DMA latency; frees the
        # wq slot early so wk(next layer) can prefetch during l=12 compute)
        for ch in range(NCHUNK):
            cs = slice(ch * CHUNK, (ch + 1) * CHUNK)
            q_ps = ps_pool.tile([128, C], F32, tag="kv", name=f"q{ch}")
            for half in range(2):
                hs = slice(half * 512, (half + 1) * 512)
                mm3(q_ps, x12_sb, wq_sb, cs, hs)
            nc.scalar.mul(q_sb[:, ch, :], q_ps[:], QSCALE)

        wv_sb = res_pool.tile([128, 2, GG, 2, C], F8, tag="wv")
        for t in range(2):
            for gg in range(GG):
                nc.sync.dma_start(wv_sb[:, t, gg], wvt[t, gg])
        id_sb = res_pool.tile([128, 128], F32, tag="id")
        nc.scalar.dma_start(id_sb[:], ident[:])

        wo_sb = None  # allocated during the last layer

        # ---- online attention over l (l=12 first, q folded in) ----
        prev = None  # (v_ps, ch, l) whose e is already requested

        def flush_prev():
            v_prev, chp, lp = prev
            m_sb = p_pool.tile([128, H, D], F32, tag="m", name=f"m_{chp}_{lp}")
            nc.vector.tensor_mul(
                m_sb[:],
                v_prev[:].rearrange("p (h d) -> p h d", d=D),
                e_all[:, chp, lp, :].unsqueeze(2).broadcast_to((128, H, D)),
            )
            nc.vector.tensor_add(num_sb[:, chp], num_sb[:, chp], m_sb[:])

        def tails():
            # Phase A: softmax normalization for every chunk (DVE) so the
            # PE phases below only ever wait for the first chunk's chain.
            att_sbs = []
            for ch in range(NCHUNK):
                den = small_pool.tile([128, H], F32, tag="den", name=f"den{ch}")
                nc.vector.tensor_reduce(
                    den[:],
                    e_all[:, ch].transpose([0, 2, 1]),
                    axis=mybir.AxisListType.X, op=mybir.AluOpType.add,
                )
                rden = small_pool.tile([128, H], F32, tag="rd", name=f"rd{ch}")
                nc.vector.reciprocal(rden[:], den[:])
                att_sb = o_pool.tile([128, H, D], F32, tag="att", name=f"att{ch}")
                nc.vector.tensor_mul(
                    att_sb[:], num_sb[:, ch],
                    rden[:].unsqueeze(2).broadcast_to((128, H, D)),
                )
                att_sbs.append(att_sb)
            # Phase B: transposes (PE) + PSUM->SBUF copies (DVE)
            attTs = []
            for ch in range(NCHUNK):
                att2 = att_sbs[ch][:].rearrange("p h d -> p (h d)")
                t_ps = ps_pool.tile([128, C], F32, tag="kv", name=f"t{ch}")
                for g in range(CI):
                    nc.tensor.transpose(
                        t_ps[:, g * 128:(g + 1) * 128],
                        att2[:, g * 128:(g + 1) * 128],
                        id_sb[:],
                    )
                attT = o_pool.tile([128, CI, 128], F32R, tag="attT",
                                   name=f"attT{ch}")
                nc.vector.tensor_copy(
                    attT[:].rearrange("p g n -> p (g n)"), t_ps[:]
                )
                attTs.append(attT)
            # Phase C: output projections; copy+store per half to shorten
            # the final drain.
            for ch in range(NCHUNK):
                o_ps = ps_pool.tile([128, C], F32, tag="kv", name=f"o{ch}")
                out_sb = o_pool.tile([128, C], F32, tag="out", name=f"out{ch}")
                for half in range(2):
                    hs = slice(half * 512, (half + 1) * 512)
                    for g in range(CI):
                        nc.tensor.matmul(
                            o_ps[:, hs], attTs[ch][:, g, :], wo_sb[:, g, hs],
                            start=(g == 0), stop=(g == CI - 1),
                        )
                    nc.scalar.copy(out_sb[:, hs], o_ps[:, hs])
                    nc.scalar.dma_start(
                        out[ch * CHUNK:(ch + 1) * CHUNK, hs], out_sb[:, hs])

        for li, l in enumerate(L_ORDER):
            is_q = (l == 12)
            last = (li == L - 1)
            if is_q:
                wk_sb, x_sb = wk12_sb, x12_sb   # preloaded at startup
            else:
                wk_sb = wk_pool.tile([128, 2, GG, 2, C], F8, tag="w",
                                     name=f"wk{l}")
                for t in range(2):
                    for gg in range(GG):
                        nc.sync.dma_start(wk_sb[:, t, gg], wkt[l, t, gg])
                x_sb = x_pool.tile([128, 2, GG, 2, NPC], F8, tag="x",
                                   name=f"x{l}")
                for t in range(2):
                    for gg in range(GG):
                        nc.scalar.dma_start(x_sb[:, t, gg], xt[l, t, gg])
            if last:
                # wo: stream in during the final layer, before the tails
                wo_sb = wk_pool.tile([128, CI, C], F32R, tag="w", name="wo")
                for g in range(CI):
                    nc.sync.dma_start(wo_sb[:, g, :], wot[g * 128:(g + 1) * 128, :])

            for ch in range(NCHUNK):
                cs = slice(ch * CHUNK, (ch + 1) * CHUNK)
                k_ps = ps_pool.tile([128, C], F32, tag="kv", name=f"k{l}_{ch}")
                v_ps = ps_pool.tile([128, C], F32, tag="kv", name=f"v{l}_{ch}")
                for half in range(2):
                    hs = slice(half * 512, (half + 1) * 512)
                    mm3(k_ps, x_sb, wk_sb, cs, hs)
                for half in range(2):
                    hs = slice(half * 512, (half + 1) * 512)
                    mm3(v_ps, x_sb, wv_sb, cs, hs)

                # scores: s[n, h] = sum_d q[n, h, d] * k[n, h, d]
                p_sb = p_pool.tile([128, H, D], F32, tag="p")
                nc.vector.tensor_mul(
                    p_sb[:],
                    q_sb[:, ch, :].rearrange("p (h d) -> p h d", d=D),
                    k_ps[:].rearrange("p (h d) -> p h d", d=D),
                )
                s_sb = small_pool.tile([128, H], F32, tag="s")
                nc.vector.tensor_reduce(
                    s_sb[:], p_sb[:], axis=mybir.AxisListType.X,
                    op=mybir.AluOpType.add,
                )
                nc.scalar.activation(
                    e_all[:, ch, l, :], s_sb[:],
                    mybir.ActivationFunctionType.Exp,
                )
                if prev is not None:
                    flush_prev()
                prev = (v_ps, ch, l)
        flush_prev()
        tails()


def _build_bass(nrep=1):
    nc = bacc.Bacc("TRN2", target_bir_lowering=False, debug=False,
                   num_devices=N_CORES)
    aps = {
        "xt": nc.dram_tensor("xt", (L, 2, GG, 128, 2, NPC), F8,
                             kind="ExternalInput").ap(),
        "chain": nc.dram_tensor("chain", (NPC, C), F32, kind="ExternalInput").ap(),
        "wkt": nc.dram_tensor("wkt", (L, 2, GG, 128, 2, C), F8,
                              kind="ExternalInput").ap(),
        "wvt": nc.dram_tensor("wvt", (2, GG, 128, 2, C), F8,
                              kind="ExternalInput").ap(),
        "wqt": nc.dram_tensor("wqt", (2, GG, 128, 2, C), F8,
                              kind="ExternalInput").ap(),
        "wot": nc.dram_tensor("wot", (C, C), F32R, kind="ExternalInput").ap(),
        "ident": nc.dram_tensor("ident", (128, 128), F32, kind="ExternalInput").ap(),
    }
    if nrep == 1:
        out = nc.dram_tensor("out", (NPC, C), F32, kind="ExternalOutput").ap()
        outs = [out]
    else:
        big = nc.dram_tensor("out", (nrep, NPC, C), F32,
                             kind="ExternalOutput").ap()
        outs = [big[r] for r in range(nrep)]
    with tile.TileContext(nc) as tc:
        for r in range(nrep):
            _emit(tc, {**aps, "out": outs[r]})
    nc.compile()
    return nc


def _rope_tables():
    inv_freq = 1.0 / (ROPE_BASE ** (np.arange(0, D, 2, dtype=np.float32) / D))
    freqs = np.arange(L, dtype=np.float32)[:, None] * inv_freq[None, :]
    emb = np.concatenate([freqs, freqs], axis=-1)          # (L, D)
    return np.cos(emb).astype(np.float32), np.sin(emb).astype(np.float32)


def _rope_weight(w, cos_l, sin_l):
    """R_l @ W for a (C, C) projection weight (rows indexed by h*D+d)."""
    w3 = w.reshape(H, D, C)
    rot = np.concatenate([-w3[:, D // 2:, :], w3[:, :D // 2, :]], axis=1)
    return (cos_l[None, :, None] * w3 + sin_l[None, :, None] * rot).reshape(C, C)


def _split8(a, scale):
    """hi/lo e4m3 split with unscaled residual (shared PSUM scale)."""
    import ml_dtypes
    e4 = ml_dtypes.float8_e4m3
    hi = (a * np.float32(scale)).astype(e4)
    lo = (a * np.float32(scale) - hi.astype(np.float32)).astype(e4)
    return hi, lo


def _pack_w(w):
    """(C_in, C_out) -> fp8 hi/lo packed (2, GG, 128, 2, C_out)."""
    hi, lo = _split8(w, SW)
    out = np.empty((2, GG, 128, 2, w.shape[1]), dtype=hi.dtype)
    for t, arr in enumerate((hi, lo)):
        # in-feature f = gg*256 + pair*128 + p
        out[t] = arr.reshape(GG, 2, 128, w.shape[1]).transpose(0, 2, 1, 3)
    return out


def _host_prep(layer_outputs, Wq, Wk, Wv, Wo):
    import ml_dtypes
    cos, sin = _rope_tables()
    wkt = np.empty((L, 2, GG, 128, 2, C), dtype=ml_dtypes.float8_e4m3)
    for l in range(L):
        wkt[l] = _pack_w(np.ascontiguousarray(_rope_weight(Wk, cos[l], sin[l]).T))
    wq12 = _rope_weight(Wq, cos[L - 1], sin[L - 1]) / np.float32(np.sqrt(D))
    shared = {
        "wkt": wkt,
        "wvt": _pack_w(np.ascontiguousarray(Wv.T)),
        "wqt": _pack_w(np.ascontiguousarray(wq12.T.astype(np.float32))),
        "wot": np.ascontiguousarray(Wo.T) / np.float32(SX * SW),
        "ident": np.eye(128, dtype=np.float32),
        "chain": np.zeros((NPC, C), dtype=np.float32),
    }
    in_maps = []
    for c in range(N_CORES):
        n0 = c * NPC
        b = n0 // T
        t0 = n0 % T
        sl = layer_outputs[:, b, t0:t0 + NPC, :]          # (L, NPC, C)
        xtf = np.ascontiguousarray(sl.transpose(0, 2, 1))  # (L, C, NPC)
        hi, lo = _split8(xtf, SX)
        x8 = np.empty((L, 2, GG, 128, 2, NPC), dtype=hi.dtype)
        for t, arr in enumerate((hi, lo)):
            x8[:, t] = arr.reshape(L, GG, 2, 128, NPC).transpose(0, 1, 3, 2, 4)
        in_maps.append({
            "xt": x8,
            **shared,
        })
    return in_maps


def _get_nc():
    global _CACHED_NC
    if _CACHED_NC is None:
        _CACHED_NC = _build_bass()
    return _CACHED_NC


def _make_runner(nc):
    """Compile-once PJRT runner for the 8-core SPMD NEFF."""
    import jax
    from jax.experimental.shard_map import shard_map
    from jax.sharding import Mesh, NamedSharding, PartitionSpec
    from concourse.bass2jax import (
        _bass_exec_p, install_neuronx_cc_hook, partition_id_tensor,
    )

    install_neuronx_cc_hook()
    partition_name = (nc.partition_id_tensor.name
                      if nc.partition_id_tensor else None)
    in_names, out_names, out_avals, zero_outs = [], [], [], []
    for alloc in nc.m.functions[0].allocations:
        if not isinstance(alloc, mybir.MemoryLocationSet):
            continue
        name = alloc.memorylocations[0].name
        if alloc.kind == "ExternalInput":
            if name != partition_name:
                in_names.append(name)
        elif alloc.kind == "ExternalOutput":
            shape = tuple(alloc.tensor_shape)
            dtype = mybir.dt.np(alloc.dtype)
            out_names.append(name)
            out_avals.append(jax.core.ShapedArray(shape, dtype))
            zero_outs.append(np.zeros(shape, dtype))
    n_params = len(in_names)
    all_in_names = list(in_names) + list(out_names)
    if partition_name is not None:
        all_in_names.append(partition_name)

    def _body(*args):
        operands = list(args)
        if partition_name is not None:
            operands.append(partition_id_tensor())
        return tuple(_bass_exec_p.bind(
            *operands,
            out_avals=tuple(out_avals),
            in_names=tuple(all_in_names),
            out_names=tuple(out_names),
            lowering_input_output_aliases=(),
            sim_require_finite=True,
            sim_require_nnan=True,
            nc=nc,
        ))

    devices = jax.devices()[:N_CORES]
    mesh = Mesh(np.asarray(devices), ("core",))
    spec = NamedSharding(mesh, PartitionSpec("core"))
    n_outs = len(out_names)
    jitted = jax.jit(
        shard_map(_body, mesh=mesh,
                  in_specs=(PartitionSpec("core"),) * (n_params + n_outs),
                  out_specs=(PartitionSpec("core"),) * n_outs,
                  check_rep=False),
        keep_unused=True,
    )

    def run(in_maps):
        import jax as _jax
        concat_in = [
            np.concatenate([np.asarray(in_maps[c][nm])
                            for c in range(N_CORES)], axis=0)
            for nm in in_names
        ]
        dev_in = [_jax.device_put(a, spec) for a in concat_in]
        zs = [_jax.device_put(
                  np.zeros((N_CORES * z.shape[0], *z.shape[1:]), z.dtype),
                  spec)
              for z in zero_outs]
        outs = jitted(*dev_in, *zs)
        _jax.block_until_ready(outs)
        full = np.asarray(outs[out_names.index("out")])
        return full  # (N_CORES*NPC, C)

    return run


def _get_runner():
    global _CACHED_RUNNER
    if _CACHED_RUNNER is None:
        _CACHED_RUNNER = _make_runner(_get_nc())
    return _CACHED_RUNNER


def kernel(layer_outputs, Wq, Wk, Wv, Wo):
    layer_outputs = np.asarray(layer_outputs, dtype=np.float32)
    Wq = np.asarray(Wq, dtype=np.float32)
    Wk = np.asarray(Wk, dtype=np.float32)
    Wv = np.asarray(Wv, dtype=np.float32)
    Wo = np.asarray(Wo, dtype=np.float32)

    in_maps = _host_prep(layer_outputs, Wq, Wk, Wv, Wo)
    full = _get_runner()(in_maps)           # (B*T, C)
    return full.reshape(B, T, C)


if __name__ == "__main__":
    nc = _build_bass()
    print("build OK")

